# revision 31
# baseline (speedup 1.0000x reference)
"""Trainium2 Bass kernel for nn_AdaptiveBilinear.

Reference computation (per batch item b, L=2048, D=512):
    a1  = softmax(x1 @ x1^T)        # (L, L)
    a2  = softmax(x2 @ x2^T)        # (L, L)
    x12 = x1 @ x2^T                 # (L, L)
    out = a1 @ x12 @ a2^T           # (L, L)

Key restructure (exact, by matmul associativity):
    out = (a1 @ x1) @ (a2 @ x2)^T = y1 @ y2^T

so each branch is a self-attention with V=X (5*L^2*D FLOPs total instead of
2*L^3 + 3*L^2*D).

Sharding: batch=8 over the 8 NeuronCores, one batch item per core; the
program is pure SPMD with no collectives.

Per-core algorithm:
    xT8 = fp8(transpose(x))            # branch1: PE transpose; branch2: DMA xbar
    diag[i] = sum_d fp8(x[i,d])^2      # matches S8's diagonal so exp() stays O(1)
    S8[j,i] = sum_d xT8[d,j] xT8[d,i]  # fp8 DoubleRow matmuls, f32 PSUM
    PT[j,i] = exp(S8[j,i] - diag[i])   # transposed unnormalized softmax, fp8.
                                       # Valid for any per-column constant; the
                                       # softmax here is saturated (~one-hot) so
                                       # fp8 logit noise does not move the output.
    sums[i] = sum_j PT[j,i]            # fp8 DoubleRow ones-matmuls -- same
                                       # quantized PT as uT, so the ratio uT/sums
                                       # cancels quantization exactly
    rs = exp(-ln(sums))                # 1/x on ScalarE (DVE reciprocal ~13us/row)
    uT[d,i] = sum_j x[j,d] PT[j,i]     # bf16 x (value precision) x fp8 PT
    yT[d,i] = uT[d,i] * rs[i]          # row-broadcast tile (GpSimd bcast)
    out[i,l] = sum_d y1T[d,i] y2T[d,l] # bf16

Engine assignment notes: every engine runs its instruction stream in order,
so branch 2's prep work (loads, casts, diag squares) is routed to ScalarE and
the DMA engines, whose stream positions leave them idle during branch 1's U
phase -- on DVE it would queue behind branch 1's subtract/multiply work and
stall the PE at the branch boundary.
"""

import numpy as np

import concourse.bass as bass
import concourse.bass_isa as bass_isa
import concourse.mybir as mybir
import concourse.tile as tile
from concourse import bacc, bass_utils
from concourse.masks import make_identity

F32 = mybir.dt.float32
BF16 = mybir.dt.bfloat16
FP8 = mybir.dt.float8e4
DR = mybir.MatmulPerfMode.DoubleRow
EXP = mybir.ActivationFunctionType.Exp
LN = mybir.ActivationFunctionType.Ln
SQUARE = mybir.ActivationFunctionType.Square

L = 2048          # sequence length per batch item
D = 512           # feature dim
NB = L // 128     # 16 row blocks
DC = D // 128     # 4 contraction chunks of 128
NC = L // 512     # 4 moving-free chunks of 512
NH = L // 1024    # 2 exp/sub chunks of 1024 per row block
N_CORES = 8


class Branch:
    def __init__(self, nc, tc, bi, sb, x_d, yT, consts):
        self.nc, self.tc, self.bi, self.sb, self.x_d = nc, tc, bi, sb, x_d
        self.yT = yT
        self.ones_col, self.ones8, self.ident, self.identf = consts
        self.xb = sb["xb"].tile([128, NB, D], BF16, tag="xb", name=f"xb{bi}")
        self.xT8 = sb["xt8"].tile([128, DC, L], FP8, tag="xT8", name=f"xT8{bi}")
        self.PT = sb["pt"].tile([128, NB, L], FP8, tag="PT", name=f"PT{bi}")
        self.MX = sb["mx"].tile([128, L], BF16, tag="MX", name=f"MX{bi}")
        self.RS = sb["rs"].tile([128, L], F32, tag="RS", name=f"RS{bi}")
        self.ndrow = sb["rows"].tile([1, L], BF16, tag="ndrow", name=f"nd{bi}")
        self.rsrow = sb["rows"].tile([1, L], F32, tag="rsrow", name=f"rs{bi}")

    # ---------------- phase A: load + cast + transpose + fp8 ----------------
    def load(self):
        nc, bi = self.nc, self.bi
        if bi == 1:
            # PE transposes: the PE is idle during the initial load phase and
            # DMA-xbar transposes would queue behind the input loads.
            with self.tc.tile_pool(name=f"ps_tp{bi}", bufs=4,
                                   space="PSUM") as ps_tp:
                for j in range(NB):
                    stg = self.sb["stage"].tile(
                        [128, D], F32, tag=f"stg{bi}", name=f"stg{bi}_{j}",
                        bufs=8)
                    r = j * 128
                    nc.sync.dma_start(stg[:64, :], self.x_d.ap()[r:r + 64, :])
                    nc.sync.dma_start(stg[64:, :],
                                      self.x_d.ap()[r + 64:r + 128, :])
                    nc.vector.tensor_copy(self.xb[:, j, :], stg[:])
                    for c in range(DC):
                        tp = ps_tp.tile([128, 128], BF16, tag="tp",
                                        name=f"tp{bi}_{j}_{c}")
                        nc.tensor.transpose(
                            tp[:], self.xb[:, j, c * 128:(c + 1) * 128],
                            self.ident[:])
                        dst = self.xT8[:, c, j * 128:(j + 1) * 128]
                        if (j + c) % 2 == 0:
                            nc.vector.tensor_copy(dst, tp[:])
                        else:
                            nc.scalar.copy(dst, tp[:])
        else:
            # DMA-xbar transposes, hidden under branch 1's compute. All casts
            # on ScalarE (idle during branch 1's U phase; DVE still owes
            # branch 1's subtracts and yT multiplies). Separate loops per
            # stage: ScalarE is in-order, so interleaving a cast that waits
            # on a DMA transpose would head-of-line-block the later casts.
            xT = self.sb["xt"].tile([128, DC, L], BF16, tag="xT",
                                    name=f"xT{bi}")
            self.xb8 = self.sb["xb8"].tile([128, NB, D], FP8, tag="xb8",
                                           name=f"xb8{bi}")
            self.diagcols = self.sb["rows"].tile([128, NB], F32,
                                                 tag="diagcols",
                                                 name=f"dcols{bi}")
            for j in range(NB):
                stg = self.sb["stage"].tile(
                    [128, D], F32, tag=f"stg{bi}", name=f"stg{bi}_{j}", bufs=2)
                r = j * 128
                nc.sync.dma_start(stg[:], self.x_d.ap()[r:r + 128, :])
                nc.scalar.copy(self.xb[:, j, :], stg[:])
            for j in range(NB):
                nc.sync.dma_start_transpose(
                    xT[:, :, j * 128:(j + 1) * 128], self.xb[:, j, :])
            for j in range(NB):
                nc.scalar.copy(self.xT8[:, :, j * 128:(j + 1) * 128],
                               xT[:, :, j * 128:(j + 1) * 128])
            # diag accumulation from natural-layout fp8 (same values as the
            # S8 diagonal): Square on ScalarE with f32 accumulate.
            for j in range(NB):
                nc.scalar.copy(self.xb8[:, j, :], self.xb[:, j, :])
                junk = self.sb["work"].tile([128, D], BF16, tag="junk",
                                            name=f"jk{bi}_{j}")
                nc.scalar.activation(junk[:], self.xb8[:, j, :], SQUARE,
                                     accum_out=self.diagcols[:, j:j + 1])

    # ---------------- phase A2: diag -> MX broadcast ----------------
    def diag(self):
        nc, bi = self.nc, self.bi
        if bi == 1:
            with self.tc.tile_pool(name=f"ps_nd{bi}", bufs=1,
                                   space="PSUM") as ps_nd:
                nd_ps = ps_nd.tile([1, L], F32, tag="nd", name=f"ndp{bi}")
                for c in range(DC):
                    for n in range(NC):
                        sq = self.sb["work"].tile(
                            [128, 512], BF16, tag="sq", name=f"sq{bi}_{c}_{n}")
                        nc.vector.tensor_mul(
                            sq[:], self.xT8[:, c, n * 512:(n + 1) * 512],
                            self.xT8[:, c, n * 512:(n + 1) * 512])
                        nc.tensor.matmul(
                            nd_ps[:, n * 512:(n + 1) * 512],
                            self.ones_col[:], sq[:],
                            start=(c == 0), stop=(c == DC - 1),
                        )
                nc.scalar.copy(self.ndrow[:], nd_ps[:])
        else:
            # One PE transpose turns the accumulated diag columns into a
            # [16,128] block; a small DMA linearizes it into the [1, L] row.
            # Needs one spare PSUM bank next to u1's pool (bufs=7 there).
            diagT = self.sb["rows"].tile([NB, 128], BF16, tag="diagT",
                                         name=f"dT{bi}")
            with self.tc.tile_pool(name=f"ps_m{bi}", bufs=1,
                                   space="PSUM") as ps_m:
                dtp = ps_m.tile([NB, 128], F32, tag="dtp", name=f"dtp{bi}")
                nc.tensor.transpose(dtp[:], self.diagcols[:], self.identf[:])
                nc.vector.tensor_copy(diagT[:], dtp[:])
            nc.sync.dma_start(self.ndrow[:], diagT[:])   # [16,128] -> [1,L]
        nc.gpsimd.partition_broadcast(self.MX[:], self.ndrow[:])

    # ---------------- phase S: S8 chunks + exp + sums ----------------
    def s_phase(self):
        nc, bi = self.nc, self.bi
        with (
            self.tc.tile_pool(name=f"ps_s{bi}", bufs=3, space="PSUM") as ps_s,
            self.tc.tile_pool(name=f"ps_sum{bi}", bufs=1,
                              space="PSUM") as ps_sum,
        ):
            for h in range(NH):
                sums_h = ps_sum.tile([1, 1024], F32, tag="sumh",
                                     name=f"sum{bi}_{h}")
                for j in range(NB):
                    sps = ps_s.tile([128, 1024], F32, tag="S",
                                    name=f"S{bi}_{j}_{h}")
                    for cp in range(DC // 2):
                        for v in range(2):
                            n = 2 * h + v
                            nc.tensor.matmul(
                                sps[:, v * 512:(v + 1) * 512],
                                self.xT8[:, 2 * cp:2 * cp + 2,
                                         j * 128:(j + 1) * 128],
                                self.xT8[:, 2 * cp:2 * cp + 2,
                                         n * 512:(n + 1) * 512],
                                start=(cp == 0), stop=(cp == DC // 2 - 1),
                                perf_mode=DR,
                            )
                    nc.vector.tensor_sub(
                        sps[:], sps[:], self.MX[:, h * 1024:(h + 1) * 1024])
                    nc.scalar.activation(
                        self.PT[:, j, h * 1024:(h + 1) * 1024], sps[:], EXP)
                    # Interleave each pair's column-sum matmuls right behind
                    # its exp; batching them at the half boundary makes the
                    # PE wait for the trailing exps in one burst.
                    if j % 2 == 1:
                        jp = j // 2
                        for v in range(2):
                            n = 2 * h + v
                            nc.tensor.matmul(
                                sums_h[:, v * 512:(v + 1) * 512],
                                self.ones8[:],
                                self.PT[:, j - 1:j + 1,
                                        n * 512:(n + 1) * 512],
                                start=(jp == 0), stop=(jp == NB // 2 - 1),
                                perf_mode=DR,
                            )
                # Copy (in every ACT table set) frees the PSUM bank; LN runs
                # once at branch end to limit Exp<->Ln table reloads.
                nc.scalar.copy(self.rsrow[:, h * 1024:(h + 1) * 1024],
                               sums_h[:])
        # rs = exp(-ln(sums)); 1/x via ScalarE
        nc.scalar.activation(self.rsrow[:], self.rsrow[:], LN)
        nc.scalar.activation(self.rsrow[:], self.rsrow[:], EXP, scale=-1.0)
        nc.gpsimd.partition_broadcast(self.RS[:], self.rsrow[:])

    # ---------------- phase U: uT accumulation + normalize ----------------
    def u_phase(self):
        nc, bi = self.nc, self.bi
        with self.tc.tile_pool(name=f"ps_u{bi}", bufs=7, space="PSUM") as ps_u:
            for c in range(DC):
                ups = [ps_u.tile([128, 512], F32, tag="u",
                                 name=f"u{bi}_{c}_{n}") for n in range(NC)]
                for j in range(NB):
                    for n in range(NC):
                        nc.tensor.matmul(
                            ups[n][:],
                            self.xb[:, j, c * 128:(c + 1) * 128],
                            self.PT[:, j, n * 512:(n + 1) * 512],
                            start=(j == 0), stop=(j == NB - 1),
                        )
                for n in range(NC):
                    nc.vector.tensor_mul(
                        self.yT[:, c, n * 512:(n + 1) * 512],
                        ups[n][:],
                        self.RS[:, n * 512:(n + 1) * 512],
                    )


def build_nc():
    nc = bacc.Bacc("TRN2", target_bir_lowering=False, debug=False,
                   num_devices=N_CORES)
    x1_d = nc.dram_tensor("x1", [L, D], F32, kind="ExternalInput")
    x2_d = nc.dram_tensor("x2", [L, D], F32, kind="ExternalInput")
    out_d = nc.dram_tensor("out", [L, L], F32, kind="ExternalOutput")

    with tile.TileContext(nc) as tc:
        with (
            tc.tile_pool(name="const", bufs=1) as constp,
            tc.tile_pool(name="ypool", bufs=1) as ypool,
            tc.tile_pool(name="xbp", bufs=2) as xbp,
            tc.tile_pool(name="xtp", bufs=1) as xtp,
            tc.tile_pool(name="xt8p", bufs=1) as xt8p,
            tc.tile_pool(name="xb8p", bufs=1) as xb8p,
            tc.tile_pool(name="ptp", bufs=1) as ptp,
            tc.tile_pool(name="mxp", bufs=1) as mxp,
            tc.tile_pool(name="rsp", bufs=1) as rsp,
            tc.tile_pool(name="rows", bufs=1) as rows,
            tc.tile_pool(name="stage", bufs=8) as stage,
            tc.tile_pool(name="work", bufs=2) as work,
            tc.tile_pool(name="osbp", bufs=3) as osbp,
        ):
            ones_col = constp.tile([128, 1], BF16, tag="ones_col")
            nc.gpsimd.memset(ones_col[:], 1.0)
            # Padded so the DoubleRow k-tile stride (16B) passes the ISA
            # alignment check; only [:, :, 0:1] is used as the weights AP.
            ones8_pad = constp.tile([128, 2, 16], FP8, tag="ones8")
            nc.gpsimd.memset(ones8_pad[:], 1.0)
            ones8 = ones8_pad[:, :, 0:1]
            ident = constp.tile([128, 128], BF16, tag="ident")
            make_identity(nc, ident[:])
            identf = constp.tile([128, 128], F32, tag="identf")
            make_identity(nc, identf[:])
            consts = (ones_col, ones8, ident, identf)

            y1T = ypool.tile([128, DC, L], BF16, tag="y1T")
            y2T = ypool.tile([128, DC, L], BF16, tag="y2T")

            sb = {"xb": xbp, "xt": xtp, "xt8": xt8p, "xb8": xb8p, "pt": ptp, "mx": mxp,
                  "rs": rsp, "rows": rows, "stage": stage, "work": work}
            b1 = Branch(nc, tc, 1, sb, x1_d, y1T, consts)
            b2 = Branch(nc, tc, 2, sb, x2_d, y2T, consts)

            # Emission order = per-engine program order. Branch 2's prep is
            # emitted early so its (GpSimd/DMA) work overlaps branch 1's
            # S/U phases instead of queueing behind them.
            b1.load()
            b1.diag()
            b1.s_phase()
            b1.u_phase()
            b2.load()
            b2.diag()
            b2.s_phase()
            b2.u_phase()

            # --- out[i,l] = sum_d y1T[d,i] y2T[d,l] ---
            with tc.tile_pool(name="ps_o", bufs=2, space="PSUM") as ps_o:
                for i in range(NB):
                    ops = ps_o.tile([128, L], F32, tag="o", name=f"o_{i}")
                    for c in range(DC):
                        for n in range(NC):
                            nc.tensor.matmul(
                                ops[:, n * 512:(n + 1) * 512],
                                y1T[:, c, i * 128:(i + 1) * 128],
                                y2T[:, c, n * 512:(n + 1) * 512],
                                start=(c == 0), stop=(c == DC - 1),
                            )
                    last = i >= NB - 2
                    for h in range(NH):
                        osb = osbp.tile([128, 1024], F32, tag="osb",
                                        name=f"osb_{i}_{h}")
                        if last:
                            # Drain the tail fast: split copies across both
                            # engines and the DMA across queues.
                            nc.scalar.copy(osb[:, :512],
                                           ops[:, h * 1024:h * 1024 + 512])
                            nc.vector.tensor_copy(
                                osb[:, 512:],
                                ops[:, h * 1024 + 512:(h + 1) * 1024])
                            for q in range(2):
                                col = h * 1024 + q * 512
                                nc.sync.dma_start(
                                    out_d.ap()[i * 128:(i + 1) * 128,
                                               col:col + 512],
                                    osb[:, q * 512:(q + 1) * 512])
                        else:
                            if h % 2 == 0:
                                nc.scalar.copy(
                                    osb[:], ops[:, h * 1024:(h + 1) * 1024])
                            else:
                                nc.vector.tensor_copy(
                                    osb[:], ops[:, h * 1024:(h + 1) * 1024])
                            nc.sync.dma_start(
                                out_d.ap()[i * 128:(i + 1) * 128,
                                           h * 1024:(h + 1) * 1024],
                                osb[:])

    nc.compile()
    return nc


_NC_CACHE = None


def _get_nc():
    global _NC_CACHE
    if _NC_CACHE is None:
        _NC_CACHE = build_nc()
    return _NC_CACHE


def kernel(x1: np.ndarray, x2: np.ndarray) -> np.ndarray:
    """Full inputs (8, 2048, 512) f32 -> full output (8, 2048, 2048) f32."""
    assert x1.shape == (N_CORES, L, D) and x2.shape == (N_CORES, L, D)
    nc = _get_nc()
    in_maps = [
        {
            "x1": np.ascontiguousarray(np.asarray(x1[b], dtype=np.float32)),
            "x2": np.ascontiguousarray(np.asarray(x2[b], dtype=np.float32)),
        }
        for b in range(N_CORES)
    ]
    res = bass_utils.run_bass_kernel_spmd(nc, in_maps, core_ids=list(range(N_CORES)))
    out = np.stack([res.results[b]["out"] for b in range(N_CORES)], axis=0)
    return out.astype(np.float32, copy=False)


if __name__ == "__main__":
    rng = np.random.default_rng(0)
    x1 = rng.standard_normal((N_CORES, L, D), dtype=np.float32)
    x2 = rng.standard_normal((N_CORES, L, D), dtype=np.float32)
    out = kernel(x1=x1, x2=x2)
    print("kernel output:", out.shape, out.dtype)


# revision 32
# speedup vs baseline: 1.1660x; 1.1660x over previous
"""Trainium2 Bass kernel for nn_AdaptiveBilinear.

Reference computation (per batch item b, L=2048, D=512):
    a1  = softmax(x1 @ x1^T)        # (L, L)
    a2  = softmax(x2 @ x2^T)        # (L, L)
    x12 = x1 @ x2^T                 # (L, L)
    out = a1 @ x12 @ a2^T           # (L, L)

Key restructure (exact, by matmul associativity):
    out = (a1 @ x1) @ (a2 @ x2)^T = y1 @ y2^T

so each branch is a self-attention with V=X (5*L^2*D FLOPs total instead of
2*L^3 + 3*L^2*D).

Sharding: batch=8 over the 8 NeuronCores, one batch item per core; the
program is pure SPMD with no collectives.

Per-core algorithm:
    xT8 = fp8(transpose(x))            # branch1: PE transpose; branch2: DMA xbar
    diag[i] = sum_d fp8(x[i,d])^2      # matches S8's diagonal so exp() stays O(1)
    S8[j,i] = sum_d xT8[d,j] xT8[d,i]  # fp8 DoubleRow matmuls, f32 PSUM
    PT[j,i] = exp(S8[j,i] - diag[i])   # transposed unnormalized softmax, fp8.
                                       # Valid for any per-column constant; the
                                       # softmax here is saturated (~one-hot) so
                                       # fp8 logit noise does not move the output.
    sums[i] = sum_j PT[j,i]            # fp8 DoubleRow ones-matmuls -- same
                                       # quantized PT as uT, so the ratio uT/sums
                                       # cancels quantization exactly
    rs = exp(-ln(sums))                # 1/x on ScalarE (DVE reciprocal ~13us/row)
    uT[d,i] = sum_j x[j,d] PT[j,i]     # bf16 x (value precision) x fp8 PT
    yT[d,i] = uT[d,i] * rs[i]          # row-broadcast tile (GpSimd bcast)
    out[i,l] = sum_d y1T[d,i] y2T[d,l] # bf16

Engine assignment notes: every engine runs its instruction stream in order,
so branch 2's prep work (loads, casts, diag squares) is routed to ScalarE and
the DMA engines, whose stream positions leave them idle during branch 1's U
phase -- on DVE it would queue behind branch 1's subtract/multiply work and
stall the PE at the branch boundary.
"""

import numpy as np

import concourse.bass as bass
import concourse.bass_isa as bass_isa
import concourse.mybir as mybir
import concourse.tile as tile
from concourse import bacc, bass_utils
from concourse.masks import make_identity

F32 = mybir.dt.float32
BF16 = mybir.dt.bfloat16
FP8 = mybir.dt.float8e4
DR = mybir.MatmulPerfMode.DoubleRow
EXP = mybir.ActivationFunctionType.Exp
LN = mybir.ActivationFunctionType.Ln
SQUARE = mybir.ActivationFunctionType.Square

L = 2048          # sequence length per batch item
D = 512           # feature dim
NB = L // 128     # 16 row blocks
DC = D // 128     # 4 contraction chunks of 128
NC = L // 512     # 4 moving-free chunks of 512
NH = L // 1024    # 2 exp/sub chunks of 1024 per row block
N_CORES = 8


class Branch:
    def __init__(self, nc, tc, bi, sb, x_d, yT, consts):
        self.nc, self.tc, self.bi, self.sb, self.x_d = nc, tc, bi, sb, x_d
        self.yT = yT
        self.ones_col, self.ones8, self.ident, self.identf = consts
        self.xb = sb["xb"].tile([128, NB, D], BF16, tag="xb", name=f"xb{bi}")
        self.xT8 = sb["xt8"].tile([128, DC, L], FP8, tag="xT8", name=f"xT8{bi}")
        self.PT = sb["pt"].tile([128, NB, L], FP8, tag="PT", name=f"PT{bi}")
        self.MX = sb["mx"].tile([128, L], BF16, tag="MX", name=f"MX{bi}")
        self.RS = sb["rs"].tile([128, L], F32, tag="RS", name=f"RS{bi}")
        self.ndrow = sb["rows"].tile([1, L], BF16, tag="ndrow", name=f"nd{bi}")
        self.rsrow = sb["rows"].tile([1, L], F32, tag="rsrow", name=f"rs{bi}")

    # ---------------- phase A: load + cast + transpose + fp8 ----------------
    def load(self):
        nc, bi = self.nc, self.bi
        if bi == 1:
            # PE transposes: the PE is idle during the initial load phase and
            # DMA-xbar transposes would queue behind the input loads.
            with self.tc.tile_pool(name=f"ps_tp{bi}", bufs=4,
                                   space="PSUM") as ps_tp:
                for j in range(NB):
                    stg = self.sb["stage"].tile(
                        [128, D], F32, tag=f"stg{bi}", name=f"stg{bi}_{j}",
                        bufs=8)
                    r = j * 128
                    nc.sync.dma_start(stg[:64, :], self.x_d.ap()[r:r + 64, :])
                    nc.sync.dma_start(stg[64:, :],
                                      self.x_d.ap()[r + 64:r + 128, :])
                    nc.vector.tensor_copy(self.xb[:, j, :], stg[:])
                    for c in range(DC):
                        tp = ps_tp.tile([128, 128], BF16, tag="tp",
                                        name=f"tp{bi}_{j}_{c}")
                        nc.tensor.transpose(
                            tp[:], self.xb[:, j, c * 128:(c + 1) * 128],
                            self.ident[:])
                        dst = self.xT8[:, c, j * 128:(j + 1) * 128]
                        if (j + c) % 2 == 0:
                            nc.vector.tensor_copy(dst, tp[:])
                        else:
                            nc.scalar.copy(dst, tp[:])
        else:
            # DMA-xbar transposes, hidden under branch 1's compute. All casts
            # on ScalarE (idle during branch 1's U phase; DVE still owes
            # branch 1's subtracts and yT multiplies). Separate loops per
            # stage: ScalarE is in-order, so interleaving a cast that waits
            # on a DMA transpose would head-of-line-block the later casts.
            xT = self.sb["xt"].tile([128, DC, L], BF16, tag="xT",
                                    name=f"xT{bi}")
            self.xb8 = self.sb["xb8"].tile([128, NB, D], FP8, tag="xb8",
                                           name=f"xb8{bi}")
            self.diagcols = self.sb["rows"].tile([128, NB], F32,
                                                 tag="diagcols",
                                                 name=f"dcols{bi}")
            for j in range(NB):
                stg = self.sb["stage"].tile(
                    [128, D], F32, tag=f"stg{bi}", name=f"stg{bi}_{j}", bufs=2)
                r = j * 128
                nc.sync.dma_start(stg[:], self.x_d.ap()[r:r + 128, :])
                nc.scalar.copy(self.xb[:, j, :], stg[:])
            for j in range(NB):
                nc.sync.dma_start_transpose(
                    xT[:, :, j * 128:(j + 1) * 128], self.xb[:, j, :])
            for j in range(NB):
                nc.scalar.copy(self.xT8[:, :, j * 128:(j + 1) * 128],
                               xT[:, :, j * 128:(j + 1) * 128])
            # diag accumulation from natural-layout fp8 (same values as the
            # S8 diagonal): Square on ScalarE with f32 accumulate.
            for j in range(NB):
                nc.scalar.copy(self.xb8[:, j, :], self.xb[:, j, :])
                junk = self.sb["work"].tile([128, D], BF16, tag="junk",
                                            name=f"jk{bi}_{j}")
                nc.scalar.activation(junk[:], self.xb8[:, j, :], SQUARE,
                                     accum_out=self.diagcols[:, j:j + 1])

    # ---------------- phase A2: diag -> MX broadcast ----------------
    def diag(self):
        nc, bi = self.nc, self.bi
        if bi == 1:
            with self.tc.tile_pool(name=f"ps_nd{bi}", bufs=1,
                                   space="PSUM") as ps_nd:
                nd_ps = ps_nd.tile([1, L], F32, tag="nd", name=f"ndp{bi}")
                for c in range(DC):
                    for n in range(NC):
                        sq = self.sb["work"].tile(
                            [128, 512], BF16, tag="sq", name=f"sq{bi}_{c}_{n}")
                        nc.vector.tensor_mul(
                            sq[:], self.xT8[:, c, n * 512:(n + 1) * 512],
                            self.xT8[:, c, n * 512:(n + 1) * 512])
                        nc.tensor.matmul(
                            nd_ps[:, n * 512:(n + 1) * 512],
                            self.ones_col[:], sq[:],
                            start=(c == 0), stop=(c == DC - 1),
                        )
                nc.scalar.copy(self.ndrow[:], nd_ps[:])
        else:
            # One PE transpose turns the accumulated diag columns into a
            # [16,128] block; a small DMA linearizes it into the [1, L] row.
            # Needs one spare PSUM bank next to u1's pool (bufs=7 there).
            diagT = self.sb["rows"].tile([NB, 128], BF16, tag="diagT",
                                         name=f"dT{bi}")
            with self.tc.tile_pool(name=f"ps_m{bi}", bufs=1,
                                   space="PSUM") as ps_m:
                dtp = ps_m.tile([NB, 128], F32, tag="dtp", name=f"dtp{bi}")
                nc.tensor.transpose(dtp[:], self.diagcols[:], self.identf[:])
                nc.vector.tensor_copy(diagT[:], dtp[:])
            nc.sync.dma_start(self.ndrow[:], diagT[:])   # [16,128] -> [1,L]
        nc.gpsimd.partition_broadcast(self.MX[:], self.ndrow[:])

    # ---------------- phase S: S8 chunks + exp + sums ----------------
    def s_phase(self):
        nc, bi = self.nc, self.bi
        with (
            self.tc.tile_pool(name=f"ps_s{bi}", bufs=3, space="PSUM") as ps_s,
            self.tc.tile_pool(name=f"ps_sum{bi}", bufs=1,
                              space="PSUM") as ps_sum,
        ):
            for h in range(NH):
                for j in range(NB):
                    sps = ps_s.tile([128, 1024], F32, tag="S",
                                    name=f"S{bi}_{j}_{h}")
                    for cp in range(DC // 2):
                        for v in range(2):
                            n = 2 * h + v
                            nc.tensor.matmul(
                                sps[:, v * 512:(v + 1) * 512],
                                self.xT8[:, 2 * cp:2 * cp + 2,
                                         j * 128:(j + 1) * 128],
                                self.xT8[:, 2 * cp:2 * cp + 2,
                                         n * 512:(n + 1) * 512],
                                start=(cp == 0), stop=(cp == DC // 2 - 1),
                                perf_mode=DR,
                            )
                    nc.vector.tensor_sub(
                        sps[:], sps[:], self.MX[:, h * 1024:(h + 1) * 1024])
                    nc.scalar.activation(
                        self.PT[:, j, h * 1024:(h + 1) * 1024], sps[:], EXP)
                sums_h = ps_sum.tile([1, 1024], F32, tag="sumh",
                                     name=f"sum{bi}_{h}")
                for jp in range(NB // 2):
                    for v in range(2):
                        n = 2 * h + v
                        nc.tensor.matmul(
                            sums_h[:, v * 512:(v + 1) * 512],
                            self.ones8[:],
                            self.PT[:, 2 * jp:2 * jp + 2,
                                    n * 512:(n + 1) * 512],
                            start=(jp == 0), stop=(jp == NB // 2 - 1),
                            perf_mode=DR,
                        )
                # Copy (in every ACT table set) frees the PSUM bank; LN runs
                # once at branch end to limit Exp<->Ln table reloads.
                nc.scalar.copy(self.rsrow[:, h * 1024:(h + 1) * 1024],
                               sums_h[:])
        # rs = exp(-ln(sums)); 1/x via ScalarE
        nc.scalar.activation(self.rsrow[:], self.rsrow[:], LN)
        nc.scalar.activation(self.rsrow[:], self.rsrow[:], EXP, scale=-1.0)
        nc.gpsimd.partition_broadcast(self.RS[:], self.rsrow[:])

    # ---------------- phase U: uT accumulation + normalize ----------------
    def u_phase(self):
        nc, bi = self.nc, self.bi
        with self.tc.tile_pool(name=f"ps_u{bi}", bufs=7, space="PSUM") as ps_u:
            for c in range(DC):
                ups = [ps_u.tile([128, 512], F32, tag="u",
                                 name=f"u{bi}_{c}_{n}") for n in range(NC)]
                for j in range(NB):
                    for n in range(NC):
                        nc.tensor.matmul(
                            ups[n][:],
                            self.xb[:, j, c * 128:(c + 1) * 128],
                            self.PT[:, j, n * 512:(n + 1) * 512],
                            start=(j == 0), stop=(j == NB - 1),
                        )
                for n in range(NC):
                    nc.vector.tensor_mul(
                        self.yT[:, c, n * 512:(n + 1) * 512],
                        ups[n][:],
                        self.RS[:, n * 512:(n + 1) * 512],
                    )


def build_nc():
    nc = bacc.Bacc("TRN2", target_bir_lowering=False, debug=False,
                   num_devices=N_CORES)
    x1_d = nc.dram_tensor("x1", [L, D], F32, kind="ExternalInput")
    x2_d = nc.dram_tensor("x2", [L, D], F32, kind="ExternalInput")
    out_d = nc.dram_tensor("out", [L, L], F32, kind="ExternalOutput")

    with tile.TileContext(nc) as tc:
        with (
            tc.tile_pool(name="const", bufs=1) as constp,
            tc.tile_pool(name="ypool", bufs=1) as ypool,
            tc.tile_pool(name="xbp", bufs=2) as xbp,
            tc.tile_pool(name="xtp", bufs=1) as xtp,
            tc.tile_pool(name="xt8p", bufs=1) as xt8p,
            tc.tile_pool(name="xb8p", bufs=1) as xb8p,
            tc.tile_pool(name="ptp", bufs=1) as ptp,
            tc.tile_pool(name="mxp", bufs=1) as mxp,
            tc.tile_pool(name="rsp", bufs=1) as rsp,
            tc.tile_pool(name="rows", bufs=1) as rows,
            tc.tile_pool(name="stage", bufs=8) as stage,
            tc.tile_pool(name="work", bufs=2) as work,
            tc.tile_pool(name="osbp", bufs=3) as osbp,
        ):
            ones_col = constp.tile([128, 1], BF16, tag="ones_col")
            nc.gpsimd.memset(ones_col[:], 1.0)
            # Padded so the DoubleRow k-tile stride (16B) passes the ISA
            # alignment check; only [:, :, 0:1] is used as the weights AP.
            ones8_pad = constp.tile([128, 2, 16], FP8, tag="ones8")
            nc.gpsimd.memset(ones8_pad[:], 1.0)
            ones8 = ones8_pad[:, :, 0:1]
            ident = constp.tile([128, 128], BF16, tag="ident")
            make_identity(nc, ident[:])
            identf = constp.tile([128, 128], F32, tag="identf")
            make_identity(nc, identf[:])
            consts = (ones_col, ones8, ident, identf)

            y1T = ypool.tile([128, DC, L], BF16, tag="y1T")
            y2T = ypool.tile([128, DC, L], BF16, tag="y2T")

            sb = {"xb": xbp, "xt": xtp, "xt8": xt8p, "xb8": xb8p, "pt": ptp, "mx": mxp,
                  "rs": rsp, "rows": rows, "stage": stage, "work": work}
            b1 = Branch(nc, tc, 1, sb, x1_d, y1T, consts)
            b2 = Branch(nc, tc, 2, sb, x2_d, y2T, consts)

            # Emission order = per-engine program order. Branch 2's prep is
            # emitted early so its (GpSimd/DMA) work overlaps branch 1's
            # S/U phases instead of queueing behind them.
            b1.load()
            b1.diag()
            b1.s_phase()
            b1.u_phase()
            b2.load()
            b2.diag()
            b2.s_phase()
            b2.u_phase()

            # --- out[i,l] = sum_d y1T[d,i] y2T[d,l] ---
            with tc.tile_pool(name="ps_o", bufs=2, space="PSUM") as ps_o:
                for i in range(NB):
                    ops = ps_o.tile([128, L], F32, tag="o", name=f"o_{i}")
                    for c in range(DC):
                        for n in range(NC):
                            nc.tensor.matmul(
                                ops[:, n * 512:(n + 1) * 512],
                                y1T[:, c, i * 128:(i + 1) * 128],
                                y2T[:, c, n * 512:(n + 1) * 512],
                                start=(c == 0), stop=(c == DC - 1),
                            )
                    last = i >= NB - 2
                    for h in range(NH):
                        osb = osbp.tile([128, 1024], F32, tag="osb",
                                        name=f"osb_{i}_{h}")
                        if last:
                            # Drain the tail fast: split copies across both
                            # engines and the DMA across queues.
                            nc.scalar.copy(osb[:, :512],
                                           ops[:, h * 1024:h * 1024 + 512])
                            nc.vector.tensor_copy(
                                osb[:, 512:],
                                ops[:, h * 1024 + 512:(h + 1) * 1024])
                            for q in range(2):
                                col = h * 1024 + q * 512
                                nc.sync.dma_start(
                                    out_d.ap()[i * 128:(i + 1) * 128,
                                               col:col + 512],
                                    osb[:, q * 512:(q + 1) * 512])
                        else:
                            if h % 2 == 0:
                                nc.scalar.copy(
                                    osb[:], ops[:, h * 1024:(h + 1) * 1024])
                            else:
                                nc.vector.tensor_copy(
                                    osb[:], ops[:, h * 1024:(h + 1) * 1024])
                            nc.sync.dma_start(
                                out_d.ap()[i * 128:(i + 1) * 128,
                                           h * 1024:(h + 1) * 1024],
                                osb[:])

    nc.compile()
    return nc


_NC_CACHE = None


def _get_nc():
    global _NC_CACHE
    if _NC_CACHE is None:
        _NC_CACHE = build_nc()
    return _NC_CACHE


def kernel(x1: np.ndarray, x2: np.ndarray) -> np.ndarray:
    """Full inputs (8, 2048, 512) f32 -> full output (8, 2048, 2048) f32."""
    assert x1.shape == (N_CORES, L, D) and x2.shape == (N_CORES, L, D)
    nc = _get_nc()
    in_maps = [
        {
            "x1": np.ascontiguousarray(np.asarray(x1[b], dtype=np.float32)),
            "x2": np.ascontiguousarray(np.asarray(x2[b], dtype=np.float32)),
        }
        for b in range(N_CORES)
    ]
    res = bass_utils.run_bass_kernel_spmd(nc, in_maps, core_ids=list(range(N_CORES)))
    out = np.stack([res.results[b]["out"] for b in range(N_CORES)], axis=0)
    return out.astype(np.float32, copy=False)


if __name__ == "__main__":
    rng = np.random.default_rng(0)
    x1 = rng.standard_normal((N_CORES, L, D), dtype=np.float32)
    x2 = rng.standard_normal((N_CORES, L, D), dtype=np.float32)
    out = kernel(x1=x1, x2=x2)
    print("kernel output:", out.shape, out.dtype)
